# revision 1
# baseline (speedup 1.0000x reference)
"""Adaptive average pooling (512,512)->(7,7) over [16,512,512,64] f32.

Data-parallel over batch: 2 samples per NeuronCore across 8 cores.
Per core: stream x[b] as [H=512, W*C=32768] f32 through SBUF in big DMAs;
stage 1 (H-window reduction) = TensorE matmul with the [H,7] pooling matrix
as lhsT (PSUM-accumulated over 4 h-blocks of 128); stage 2 (W-window
reduction) = VectorE strided reduce_sum straight out of PSUM into the
[7, 7*64] output tile. The full 1/(74*74) normalization is folded into the
stage-1 weights. Memory-bound: 128 MiB HBM read per core.
"""

import numpy as np

import concourse.mybir as mybir
from concourse import bacc
from concourse.tile import TileContext
from concourse.bass_utils import run_bass_kernel_spmd

B, H, W, C = 16, 512, 512, 64
OH = OW = 7
N_CORES = 8
BPC = B // N_CORES          # samples per core
WC = W * C                  # 32768 f32 columns per h-row
P = 128                     # SBUF partitions
HB = H // P                 # 4 h-blocks
CH = 2048                   # wc columns per input DMA tile (4 MiB tiles)
GRP = 1024                  # wc columns per PSUM accumulation group (16 w)
F32 = mybir.dt.float32


def _pool_windows(in_size: int, out_size: int):
    """Same window math as the reference _pool_matrix."""
    o = np.arange(out_size, dtype=np.float32)
    start = (o * in_size / out_size).astype(np.int32)
    stop = np.ceil((o + 1) * in_size / out_size).astype(np.int32)
    return [(int(s), int(e)) for s, e in zip(start, stop)]


def _stage1_weights() -> np.ndarray:
    """[H, OH] f32: wt[h, oh] = 1/(len_h * len_w) inside h-window oh, else 0."""
    h_windows = _pool_windows(H, OH)
    # all W windows have length 74 here; fold the W normalization in too
    w_len = float(_pool_windows(W, OW)[0][1] - _pool_windows(W, OW)[0][0])
    wt = np.zeros((H, OH), dtype=np.float64)
    for oh, (s, e) in enumerate(h_windows):
        wt[s:e, oh] = 1.0 / ((e - s) * w_len)
    return wt.astype(np.float32)


def _build():
    nc = bacc.Bacc(None, target_bir_lowering=False)
    x = nc.dram_tensor("x", [BPC, H, W, C], F32, kind="ExternalInput")
    wt = nc.dram_tensor("wt", [H, OH], F32, kind="ExternalInput")
    out = nc.dram_tensor("out", [BPC, OH, OW, C], F32, kind="ExternalOutput")

    # [b, p, hb, wc]: partition = h within block, free = (h-block, wc)
    xv = x[:].rearrange("b (hb p) w c -> b p hb (w c)", p=P)
    wv = wt[:].rearrange("(hb p) o -> p hb o", p=P)
    ov = out[:].rearrange("b oh ow c -> b oh (ow c)")

    w_windows = _pool_windows(W, OW)
    w_per_grp = GRP // C  # 16 w positions per PSUM group

    with TileContext(nc) as tc:
        with (
            tc.tile_pool(name="const", bufs=1) as const,
            tc.tile_pool(name="xin", bufs=4) as xin,
            tc.tile_pool(name="psum", bufs=4, space="PSUM") as psum,
            tc.tile_pool(name="tmp", bufs=4) as tmp,
            tc.tile_pool(name="yout", bufs=BPC) as yout,
        ):
            wt_sb = const.tile([P, HB, OH], F32)
            nc.scalar.dma_start(out=wt_sb[:], in_=wv)

            y_tiles = []
            for b in range(BPC):
                y = yout.tile([OH, OW * C], F32, tag=f"y{b}")
                y_tiles.append(y)
                started = [False] * OW
                for j in range(WC // CH):
                    xt = xin.tile([P, HB, CH], F32)
                    nc.sync.dma_start(
                        out=xt[:], in_=xv[b, :, :, j * CH : (j + 1) * CH]
                    )
                    for g in range(CH // GRP):
                        ps = psum.tile([OH, GRP], F32)
                        for q in range(GRP // 512):
                            col = g * GRP + q * 512
                            for hb in range(HB):
                                nc.tensor.matmul(
                                    ps[:, q * 512 : (q + 1) * 512],
                                    wt_sb[:, hb, :],
                                    xt[:, hb, col : col + 512],
                                    start=(hb == 0),
                                    stop=(hb == HB - 1),
                                )
                        # stage 2: this group covers w in [w0, w0+16)
                        w0 = (j * CH + g * GRP) // C
                        for ow, (ws, we) in enumerate(w_windows):
                            s = max(ws, w0)
                            e = min(we, w0 + w_per_grp)
                            if s >= e:
                                continue
                            seg = ps[:, (s - w0) * C : (e - w0) * C].rearrange(
                                "p (w c) -> p c w", c=C
                            )
                            yslice = y[:, ow * C : (ow + 1) * C]
                            if not started[ow]:
                                nc.vector.reduce_sum(
                                    yslice, seg, axis=mybir.AxisListType.X
                                )
                                started[ow] = True
                            else:
                                t = tmp.tile([OH, C], F32)
                                nc.vector.reduce_sum(
                                    t[:], seg, axis=mybir.AxisListType.X
                                )
                                nc.vector.tensor_add(out=yslice, in0=yslice, in1=t[:])
            for b in range(BPC):
                nc.scalar.dma_start(out=ov[b], in_=y_tiles[b][:])
    nc.finalize()
    return nc


_NC_CACHE = []


def kernel(x: np.ndarray) -> np.ndarray:
    assert x.shape == (B, H, W, C), x.shape
    x = np.ascontiguousarray(x, dtype=np.float32)
    wt = _stage1_weights()
    if not _NC_CACHE:
        _NC_CACHE.append(_build())
    nc = _NC_CACHE[0]
    in_maps = [
        {"x": x[i * BPC : (i + 1) * BPC], "wt": wt} for i in range(N_CORES)
    ]
    res = run_bass_kernel_spmd(nc, in_maps, core_ids=list(range(N_CORES)))
    return np.concatenate([res.results[i]["out"] for i in range(N_CORES)], axis=0)


# revision 2
# speedup vs baseline: 1.1034x; 1.1034x over previous
"""Adaptive average pooling (512,512)->(7,7) over [16,512,512,64] f32.

Data-parallel over batch: 2 samples per NeuronCore across 8 cores.
Per core: stream x[b] as [H=512, W*C=32768] through SBUF in big DMAs,
casting f32->bf16 in-flight (SWDGE) so the TensorEngine runs at full rate;
stage 1 (H-window reduction) = TensorE matmul with an exact 0/1 bf16
window mask as lhsT (PSUM f32-accumulated over 4 h-blocks of 128);
stage 2 (W-window reduction) = VectorE strided reduce_sum straight out of
PSUM into the [7, 7*64] output tile; one final f32 scale by 1/(74*74).
Memory-bound: 128 MiB HBM read per core.
"""

import numpy as np

import concourse.mybir as mybir
from concourse import bacc
from concourse.tile import TileContext
from concourse.bass_utils import run_bass_kernel_spmd

B, H, W, C = 16, 512, 512, 64
OH = OW = 7
N_CORES = 8
BPC = B // N_CORES          # samples per core
WC = W * C                  # 32768 f32 columns per h-row
P = 128                     # SBUF partitions
HB = H // P                 # 4 h-blocks
CH = 4096                   # wc columns per input DMA tile (8 MiB f32 read)
GRP = 1024                  # wc columns per PSUM accumulation group (16 w)
F32 = mybir.dt.float32
BF16 = mybir.dt.bfloat16


def _pool_windows(in_size: int, out_size: int):
    """Same window math as the reference _pool_matrix."""
    o = np.arange(out_size, dtype=np.float32)
    start = (o * in_size / out_size).astype(np.int32)
    stop = np.ceil((o + 1) * in_size / out_size).astype(np.int32)
    return [(int(s), int(e)) for s, e in zip(start, stop)]


def _mask_weights() -> np.ndarray:
    """[H, OH] f32 0/1 mask of the H pooling windows (exact in bf16)."""
    wt = np.zeros((H, OH), dtype=np.float32)
    for oh, (s, e) in enumerate(_pool_windows(H, OH)):
        wt[s:e, oh] = 1.0
    return wt


def _out_scale() -> float:
    h_len = _pool_windows(H, OH)[0][1] - _pool_windows(H, OH)[0][0]
    w_len = _pool_windows(W, OW)[0][1] - _pool_windows(W, OW)[0][0]
    return 1.0 / (h_len * w_len)


def _build():
    nc = bacc.Bacc(None, target_bir_lowering=False)
    x = nc.dram_tensor("x", [BPC, H, W, C], F32, kind="ExternalInput")
    wt = nc.dram_tensor("wt", [H, OH], F32, kind="ExternalInput")
    out = nc.dram_tensor("out", [BPC, OH, OW, C], F32, kind="ExternalOutput")

    # [b, p, hb, wc]: partition = h within block, free = (h-block, wc)
    xv = x[:].rearrange("b (hb p) w c -> b p hb (w c)", p=P)
    wv = wt[:].rearrange("(hb p) o -> p hb o", p=P)
    ov = out[:].rearrange("b oh ow c -> b oh (ow c)")

    w_windows = _pool_windows(W, OW)
    w_per_grp = GRP // C  # 16 w positions per PSUM group

    with TileContext(nc) as tc:
        with (
            tc.tile_pool(name="const", bufs=1) as const,
            tc.tile_pool(name="xin", bufs=3) as xin,
            tc.tile_pool(name="psum", bufs=4, space="PSUM") as psum,
            tc.tile_pool(name="tmp", bufs=4) as tmp,
            tc.tile_pool(name="yout", bufs=BPC) as yout,
        ):
            wt_sb = const.tile([P, HB, OH], BF16)
            nc.gpsimd.dma_start(out=wt_sb[:], in_=wv)  # f32 -> bf16 cast

            y_tiles = []
            for b in range(BPC):
                y = yout.tile([OH, OW * C], F32, tag=f"y{b}")
                y_tiles.append(y)
                started = [False] * OW
                for j in range(WC // CH):
                    xt = xin.tile([P, HB, CH], BF16)
                    nc.gpsimd.dma_start(  # f32 -> bf16 cast in flight
                        out=xt[:], in_=xv[b, :, :, j * CH : (j + 1) * CH]
                    )
                    for g in range(CH // GRP):
                        ps = psum.tile([OH, GRP], F32)
                        for q in range(GRP // 512):
                            col = g * GRP + q * 512
                            for hb in range(HB):
                                nc.tensor.matmul(
                                    ps[:, q * 512 : (q + 1) * 512],
                                    wt_sb[:, hb, :],
                                    xt[:, hb, col : col + 512],
                                    start=(hb == 0),
                                    stop=(hb == HB - 1),
                                )
                        # stage 2: this group covers w in [w0, w0+16)
                        w0 = (j * CH + g * GRP) // C
                        for ow, (ws, we) in enumerate(w_windows):
                            s = max(ws, w0)
                            e = min(we, w0 + w_per_grp)
                            if s >= e:
                                continue
                            seg = ps[:, (s - w0) * C : (e - w0) * C].rearrange(
                                "p (w c) -> p c w", c=C
                            )
                            yslice = y[:, ow * C : (ow + 1) * C]
                            if not started[ow]:
                                nc.vector.reduce_sum(
                                    yslice, seg, axis=mybir.AxisListType.X
                                )
                                started[ow] = True
                            else:
                                t = tmp.tile([OH, C], F32)
                                nc.vector.reduce_sum(
                                    t[:], seg, axis=mybir.AxisListType.X
                                )
                                nc.vector.tensor_add(out=yslice, in0=yslice, in1=t[:])
                nc.scalar.mul(y[:], y[:], _out_scale())
            for b in range(BPC):
                nc.scalar.dma_start(out=ov[b], in_=y_tiles[b][:])
    nc.finalize()
    return nc


_NC_CACHE = []


def kernel(x: np.ndarray) -> np.ndarray:
    assert x.shape == (B, H, W, C), x.shape
    x = np.ascontiguousarray(x, dtype=np.float32)
    wt = _mask_weights()
    if not _NC_CACHE:
        _NC_CACHE.append(_build())
    nc = _NC_CACHE[0]
    in_maps = [
        {"x": x[i * BPC : (i + 1) * BPC], "wt": wt} for i in range(N_CORES)
    ]
    res = run_bass_kernel_spmd(nc, in_maps, core_ids=list(range(N_CORES)))
    return np.concatenate([res.results[i]["out"] for i in range(N_CORES)], axis=0)


# revision 6
# speedup vs baseline: 1.2253x; 1.1104x over previous
"""Adaptive average pooling (512,512)->(7,7) over [16,512,512,64] f32.

Data-parallel over batch: 2 samples per NeuronCore across 8 cores.
Per core: stream x[b] as [H=512, W*C=32768] through SBUF in big DMAs,
casting f32->bf16 in-flight (SWDGE) so the TensorEngine runs at full rate;
stage 1 (H-window reduction) = TensorE matmul with an exact 0/1 bf16
window mask as lhsT (PSUM f32-accumulated over 4 h-blocks of 128);
stage 2 (W-window reduction) = VectorE strided reduce_sum straight out of
PSUM into the [7, 7*64] output tile; one final f32 scale by 1/(74*74).
Memory-bound: 128 MiB HBM read per core.
"""

import numpy as np

import concourse.mybir as mybir
from concourse import bacc
from concourse.tile import TileContext
from concourse.bass_utils import run_bass_kernel_spmd

B, H, W, C = 16, 512, 512, 64
OH = OW = 7
N_CORES = 8
BPC = B // N_CORES          # samples per core
WC = W * C                  # 32768 f32 columns per h-row
P = 128                     # SBUF partitions
HB = H // P                 # 4 h-blocks
CH = 4096                   # wc columns per chunk (4 x 2 MiB f32 DMAs, one per h-block)
GRP = 2048                  # wc columns per PSUM accumulation group (32 w, 4 banks)
F32 = mybir.dt.float32
BF16 = mybir.dt.bfloat16


def _pool_windows(in_size: int, out_size: int):
    """Same window math as the reference _pool_matrix."""
    o = np.arange(out_size, dtype=np.float32)
    start = (o * in_size / out_size).astype(np.int32)
    stop = np.ceil((o + 1) * in_size / out_size).astype(np.int32)
    return [(int(s), int(e)) for s, e in zip(start, stop)]


def _mask_weights() -> np.ndarray:
    """[H, OH] f32 0/1 mask of the H pooling windows (exact in bf16)."""
    wt = np.zeros((H, OH), dtype=np.float32)
    for oh, (s, e) in enumerate(_pool_windows(H, OH)):
        wt[s:e, oh] = 1.0
    return wt


def _out_scale() -> float:
    h_len = _pool_windows(H, OH)[0][1] - _pool_windows(H, OH)[0][0]
    w_len = _pool_windows(W, OW)[0][1] - _pool_windows(W, OW)[0][0]
    return 1.0 / (h_len * w_len)


def _build():
    nc = bacc.Bacc(None, target_bir_lowering=False)
    x = nc.dram_tensor("x", [BPC, H, W, C], F32, kind="ExternalInput")
    wt = nc.dram_tensor("wt", [H, OH], F32, kind="ExternalInput")
    out = nc.dram_tensor("out", [BPC, OH, OW, C], F32, kind="ExternalOutput")

    # [b, hb, p, wc]: partition = h within block; one DMA per (b, hb, chunk)
    xv = x[:].rearrange("b (hb p) w c -> b hb p (w c)", p=P)
    wv = wt[:].rearrange("(hb p) o -> p hb o", p=P)
    ov = out[:].rearrange("b oh ow c -> b oh (ow c)")

    w_windows = _pool_windows(W, OW)
    w_per_grp = GRP // C  # 16 w positions per PSUM group

    with TileContext(nc) as tc:
        with (
            tc.tile_pool(name="const", bufs=1) as const,
            tc.tile_pool(name="xin", bufs=12) as xin,
            tc.tile_pool(name="psum", bufs=2, space="PSUM") as psum,
            tc.tile_pool(name="tmp", bufs=4) as tmp,
            tc.tile_pool(name="yout", bufs=BPC) as yout,
        ):
            wt_sb = const.tile([P, HB, OH], BF16)
            nc.gpsimd.dma_start(out=wt_sb[:], in_=wv)  # f32 -> bf16 cast

            y_tiles = []
            for b in range(BPC):
                y = yout.tile([OH, OW * C], F32, tag=f"y{b}")
                y_tiles.append(y)
                started = [False] * OW
                for j in range(WC // CH):
                    xts = []
                    for hb in range(HB):
                        xt = xin.tile([P, CH], BF16, tag="xt")
                        nc.gpsimd.dma_start(  # f32 -> bf16 cast in flight
                            out=xt[:], in_=xv[b, hb, :, j * CH : (j + 1) * CH]
                        )
                        xts.append(xt)
                    for g in range(CH // GRP):
                        ps = psum.tile([OH, GRP], F32)
                        for q in range(GRP // 512):
                            col = g * GRP + q * 512
                            for hb in range(HB):
                                nc.tensor.matmul(
                                    ps[:, q * 512 : (q + 1) * 512],
                                    wt_sb[:, hb, :],
                                    xts[hb][:, col : col + 512],
                                    start=(hb == 0),
                                    stop=(hb == HB - 1),
                                )
                        # stage 2: this group covers w in [w0, w0+16)
                        w0 = (j * CH + g * GRP) // C
                        for ow, (ws, we) in enumerate(w_windows):
                            s = max(ws, w0)
                            e = min(we, w0 + w_per_grp)
                            if s >= e:
                                continue
                            seg = ps[:, (s - w0) * C : (e - w0) * C].rearrange(
                                "p (w c) -> p c w", c=C
                            )
                            yslice = y[:, ow * C : (ow + 1) * C]
                            if not started[ow]:
                                nc.vector.reduce_sum(
                                    yslice, seg, axis=mybir.AxisListType.X
                                )
                                started[ow] = True
                            else:
                                t = tmp.tile([OH, C], F32)
                                nc.vector.reduce_sum(
                                    t[:], seg, axis=mybir.AxisListType.X
                                )
                                nc.vector.tensor_add(out=yslice, in0=yslice, in1=t[:])
                nc.scalar.mul(y[:], y[:], _out_scale())
            for b in range(BPC):
                nc.scalar.dma_start(out=ov[b], in_=y_tiles[b][:])
    nc.finalize()
    return nc


_NC_CACHE = []


def kernel(x: np.ndarray) -> np.ndarray:
    assert x.shape == (B, H, W, C), x.shape
    x = np.ascontiguousarray(x, dtype=np.float32)
    wt = _mask_weights()
    if not _NC_CACHE:
        _NC_CACHE.append(_build())
    nc = _NC_CACHE[0]
    in_maps = [
        {"x": x[i * BPC : (i + 1) * BPC], "wt": wt} for i in range(N_CORES)
    ]
    res = run_bass_kernel_spmd(nc, in_maps, core_ids=list(range(N_CORES)))
    return np.concatenate([res.results[i]["out"] for i in range(N_CORES)], axis=0)
